# revision 22
# baseline (speedup 1.0000x reference)
"""Trainium2 Bass kernel for nn_Encoder_55490977464569 (binary-tree GRU encoder).

Strategy (v3)
-------------
Data-parallel over batch: B=16 -> 2 batch columns per NeuronCore, zero
collectives. Each core runs its whole tree (32767 nodes) leaves->root with all
hidden states resident in SBUF (bf16); only `targets` is streamed in.

Layout: feature-major [128 features (partitions), node*batch columns], each
level's nodes stored in BIT-REVERSED in-level order. With bit-reversal at
every level, the children of parent tile [t0, t0+T) are planeL =
child[:, t0:t0+T] and planeR = child[:, R_parent+t0 : +T] (both contiguous),
and the parent's h is written back contiguously. All elementwise ops run on
packed bf16 SBUF operands (DVE 2x mode); STT (1x) and gpsimd (SBUF port
contention) are avoided entirely -- measured on HW: TT=2x, TENSOR_SCALAR=4x,
STT=1x, gpsimd TT inflates concurrent DVE ops 3.6x.

Gates use Sigmoid ACTs (plain weights); 1-z comes free via ACT scale=-1:
zz = sigmoid(-z_pre). Per interior tile pair (2 tiles of T parents):
  - per tile: one [128,T] x DMA (replicated 4x across partition strips);
    4 tile_position-packed K=32 matmuls run concurrently seeding
    ps_r[0:T], ps_r[T:2T], ps_z, ps_n; W_hr @ hl|hr accumulates into ps_r;
    one Sigmoid ACT [2T] -> r; one TT [2T] with a two-region child AP
    computes t2 = r * h_child for both children.
  - per pair: cs = hl+hr as one [2T] TT; W_hz @ cs and W_hn @ t2l + t2r
    accumulate into ps_zn; zz = sigmoid(-ps_z) and n = tanh(ps_n) ACTs;
    blend h = cs + zz*(n - cs) as three [2T]-wide TTs over two-region APs.
Emission is software-pipelined one unit deep so the PE streams continuously.
"""

import sys

if "/opt/trn_rl_repo" not in sys.path:
    sys.path.insert(0, "/opt/trn_rl_repo")
if "/opt/trn_rl_repo/concourse" not in sys.path:
    sys.path.insert(0, "/opt/trn_rl_repo/concourse")

import numpy as np
import ml_dtypes

from concourse import bass, mybir, tile, bacc
from concourse import bass_utils

BF16NP = ml_dtypes.bfloat16
F32 = mybir.dt.float32
BF16 = mybir.dt.bfloat16

N_CORES = 8
DEPTH = 15
HID = 128
IN_DIM = 32
OUT_DIM = 64
BATCH = 16
B_LOCAL = BATCH // N_CORES

T_TILE = 512
SMALL_MAX_LVL = 7

ADD = mybir.AluOpType.add
SUB = mybir.AluOpType.subtract
MULT = mybir.AluOpType.mult
TANH = mybir.ActivationFunctionType.Tanh
SIGM = mybir.ActivationFunctionType.Sigmoid


def _R(l):
    return 2**l * B_LOCAL


def _bitrev(n_bits):
    n = 1 << n_bits
    p = np.zeros(n, dtype=np.int64)
    for i in range(n):
        r = 0
        x = i
        for _ in range(n_bits):
            r = (r << 1) | (x & 1)
            x >>= 1
        p[i] = r
    return p


def build_program(with_mask=False, with_bias=False):
    nc = bacc.Bacc("TRN2", target_bir_lowering=False, debug=False,
                   num_devices=1)
    leaf = DEPTH - 1

    int_lvls = list(range(DEPTH - 2, SMALL_MAX_LVL, -1))
    int_off = {}
    off = 0
    for l in int_lvls:
        int_off[l] = off
        off += _R(l)
    xint_d = nc.dram_tensor("xint", [128, off], BF16, kind="ExternalInput")
    n_pairs = _R(leaf) // (2 * T_TILE)
    xleaf_d = nc.dram_tensor("xleaf", [128, n_pairs * T_TILE], BF16,
                             kind="ExternalInput")
    small_cols = sum(_R(l) for l in range(SMALL_MAX_LVL + 1))
    xsmall_d = nc.dram_tensor("xsmall", [128, small_cols], BF16,
                              kind="ExternalInput")
    wcat_d = nc.dram_tensor("wcat", [128, 5 * HID], BF16, kind="ExternalInput")
    w_out_d = nc.dram_tensor("w_out", [HID, 2 * OUT_DIM], F32,
                             kind="ExternalInput")
    out_d = nc.dram_tensor("out", [HID, B_LOCAL], F32, kind="ExternalOutput")
    if with_bias:
        bias_d = nc.dram_tensor("biases", [HID, 4], F32, kind="ExternalInput")
    if with_mask:
        total_z = sum(_R(l) for l in range(DEPTH))
        mask_d = nc.dram_tensor("mask_bc", [HID, total_z], BF16,
                                kind="ExternalInput")
        mask_off = {}
        moff = 0
        for l in range(DEPTH):
            mask_off[l] = moff
            moff += _R(l)

    from contextlib import ExitStack
    with tile.TileContext(nc) as tc, ExitStack() as stack:
        consts = stack.enter_context(tc.tile_pool(name="consts", bufs=1))
        hpool = stack.enter_context(tc.tile_pool(name="hpool", bufs=1))
        xpool = stack.enter_context(tc.tile_pool(name="xpool", bufs=6))
        apool = stack.enter_context(tc.tile_pool(name="apool", bufs=4))
        tpool = stack.enter_context(tc.tile_pool(name="tpool", bufs=4))
        pspool = stack.enter_context(tc.tile_pool(name="pspool", bufs=2,
                                                  space="PSUM"))
        opool = stack.enter_context(tc.tile_pool(name="opool", bufs=1))

        wcat_sb = consts.tile([128, 5 * HID], BF16, name="wcat_sb",
                              tag="wcat_sb")
        nc.sync.dma_start(out=wcat_sb, in_=wcat_d.ap())
        w_hr = wcat_sb[:, 0 * HID:1 * HID]
        w_hz = wcat_sb[:, 1 * HID:2 * HID]
        w_hn = wcat_sb[:, 2 * HID:3 * HID]
        wx = wcat_sb[:, 3 * HID:4 * HID]   # [w_ir; w_ir; w_iz; w_in]
        wl = wcat_sb[:, 4 * HID:5 * HID]   # [w_iz; w_in; w_iz; w_in]
        w_out = consts.tile([HID, 2 * OUT_DIM], F32, name="w_out_sb",
                            tag="w_out_sb")
        xsmall = consts.tile([128, small_cols], BF16, name="xsmall",
                             tag="xsmall")
        # (their DMAs are emitted after the first leaf units, off the
        # startup critical path)
        small_off = {}
        soff = 0
        for l in range(SMALL_MAX_LVL, -1, -1):
            small_off[l] = soff
            soff += _R(l)
        if with_bias:
            bias_sb = consts.tile([HID, 4], F32, name="bias_sb", tag="bias_sb")
            nc.sync.dma_start(out=bias_sb, in_=bias_d.ap())
            b_r = bias_sb[:, 0:1]
            b_zneg = bias_sb[:, 1:2]   # -(b_iz + b_hz)
            b_n = bias_sb[:, 2:3]
            b_out = bias_sb[:, 3:4]

        h_lvl = [hpool.tile([HID, _R(l)], BF16, name=f"h_{l}", tag=f"h_{l}")
                 for l in range(DEPTH)]

        def mask_mul(view, lvl, col0, width):
            m_sb = tpool.tile([HID, width], BF16, name="m_sb", tag="m_sb")
            nc.sync.dma_start(
                out=m_sb,
                in_=mask_d.ap()[:, mask_off[lvl] + col0:
                                mask_off[lvl] + col0 + width])
            nc.vector.tensor_mul(view, view, m_sb)

        def act_zz(dst, src):
            if with_bias:
                nc.scalar.activation(dst, src, SIGM, bias=b_zneg, scale=-1.0)
            else:
                nc.scalar.activation(dst, src, SIGM, scale=-1.0)

        def act_n(dst, src):
            if with_bias:
                nc.scalar.activation(dst, src, TANH, bias=b_n)
            else:
                nc.scalar.activation(dst, src, TANH)

        # ---------------- leaf pair-tiles ----------------
        def leaf_front(k):
            T = T_TILE
            xp = xpool.tile([128, T], BF16, name="xp_leaf", tag="xp")
            nc.sync.dma_start(out=xp, in_=xleaf_d.ap()[:, k * T:(k + 1) * T])
            psA = pspool.tile([HID, 1024], F32, name="psA", tag="psr")
            psB = pspool.tile([HID, 1024], F32, name="psB", tag="psz")
            for s, dst in enumerate((psA[:, 0:T], psA[:, 512:512 + T],
                                     psB[:, 0:T], psB[:, 512:512 + T])):
                nc.tensor.matmul(dst, wl[32 * s:32 * (s + 1)],
                                 xp[32 * s:32 * (s + 1)],
                                 start=True, stop=True,
                                 tile_position=(32 * s, 0))
            znA = apool.tile([HID, 2 * T], BF16, name="znA", tag="act")
            znB = apool.tile([HID, 2 * T], BF16, name="znB", tag="act")
            for ps, zn in ((psA, znA), (psB, znB)):
                act_zz(zn[:, 0:T], ps[:, 0:T])
                act_n(zn[:, T:2 * T], ps[:, 512:512 + T])
            return {"znA": znA, "znB": znB, "k": k}

        def leaf_back(st):
            T = T_TILE
            k = st["k"]
            znA, znB = st["znA"], st["znB"]
            hv = h_lvl[leaf]
            half = _R(leaf) // 2
            nc.vector.tensor_mul(hv[:, k * T:(k + 1) * T],
                                 znA[:, 0:T], znA[:, T:2 * T])
            nc.vector.tensor_mul(hv[:, half + k * T:half + (k + 1) * T],
                                 znB[:, 0:T], znB[:, T:2 * T])
            if with_mask:
                mask_mul(hv[:, k * T:(k + 1) * T], leaf, k * T, T)
                mask_mul(hv[:, half + k * T:half + (k + 1) * T], leaf,
                         half + k * T, T)

        # ---------------- interior tiles ----------------
        def int_front(l, k, T, cs_width=0, split_r=False):
            """r-phase of one tile. cs_width>0: also emit the cs add
            (hl+hr) covering [t0, t0+cs_width) -- off the critical chain,
            it only needs the child level."""
            t0 = k * T
            ch = h_lvl[l + 1]
            R = _R(l)
            hoff = max(T, 512)
            hl = ch[:, t0:t0 + T]
            hr = ch[:, R + t0:R + t0 + T]
            if l > SMALL_MAX_LVL:
                xp = xpool.tile([128, T], BF16, name="xp_int", tag="xp")
                nc.sync.dma_start(
                    out=xp, in_=xint_d.ap()[:, int_off[l] + t0:
                                            int_off[l] + t0 + T])
            else:
                xp = xsmall[:, small_off[l] + t0:small_off[l] + t0 + T]
            cs = None
            if cs_width:
                cs = tpool.tile([HID, cs_width], BF16, name="cs", tag="cs")
                nc.vector.tensor_add(cs, ch[:, t0:t0 + cs_width],
                                     ch[:, R + t0:R + t0 + cs_width])
            ps_r = pspool.tile([HID, 1024], F32, name="ps_r", tag="psr")
            for s, dst in ((0, ps_r[:, 0:T]), (1, ps_r[:, hoff:hoff + T])):
                nc.tensor.matmul(dst, wx[32 * s:32 * (s + 1)],
                                 xp[32 * s:32 * (s + 1)],
                                 start=True, stop=False,
                                 tile_position=(32 * s, 0))
            nc.tensor.matmul(ps_r[:, 0:T], w_hr, hl, start=False, stop=True)
            nc.tensor.matmul(ps_r[:, hoff:hoff + T], w_hr, hr,
                             start=False, stop=True)
            r_sb = apool.tile([HID, 2 * T], BF16, name="r_sb", tag="act")
            t2 = tpool.tile([HID, 2 * T], BF16, name="t2", tag="t2")
            kw = dict(bias=b_r) if with_bias else {}
            if T == hoff and not split_r:
                nc.scalar.activation(r_sb, ps_r, SIGM, **kw)
                # t2 = r * h_child, both children via one 2-region child AP
                ch2 = ch.rearrange("p (g f) -> p g f", g=2)[:, :, t0:t0 + T]
                nc.vector.tensor_mul(t2.rearrange("p (g f) -> p g f", g=2),
                                     r_sb.rearrange("p (g f) -> p g f", g=2),
                                     ch2)
            else:
                # latency-split: t2l can proceed while rr still activates
                nc.scalar.activation(r_sb[:, 0:T], ps_r[:, 0:T], SIGM, **kw)
                nc.vector.tensor_mul(t2[:, 0:T], r_sb[:, 0:T], hl)
                nc.scalar.activation(r_sb[:, T:2 * T],
                                     ps_r[:, hoff:hoff + T], SIGM, **kw)
                nc.vector.tensor_mul(t2[:, T:2 * T], r_sb[:, T:2 * T], hr)
            return {"l": l, "t0": t0, "T": T, "t2": t2, "xp": xp, "cs": cs}

        def pair_back(stA, stB):
            """zn-phase for two adjacent tiles (t0 of B = t0 of A + T)."""
            l, T = stA["l"], stA["T"]
            t0 = stA["t0"]
            hoff = max(T, 512)
            cs = stA["cs"]
            znb = apool.tile([HID, 4 * T], BF16, name="znb", tag="znb")
            for i, st in enumerate((stA, stB)):
                t2, xp = st["t2"], st["xp"]
                ps_zn = pspool.tile([HID, 1024], F32, name="ps_zn", tag="psz")
                for s, dst in ((2, ps_zn[:, 0:T]),
                               (3, ps_zn[:, hoff:hoff + T])):
                    nc.tensor.matmul(dst, wx[32 * s:32 * (s + 1)],
                                     xp[32 * s:32 * (s + 1)],
                                     start=True, stop=False,
                                     tile_position=(32 * s, 0))
                nc.tensor.matmul(ps_zn[:, 0:T], w_hz, cs[:, i * T:(i + 1) * T],
                                 start=False, stop=True)
                nc.tensor.matmul(ps_zn[:, hoff:hoff + T], w_hn, t2[:, 0:T],
                                 start=False, stop=False)
                nc.tensor.matmul(ps_zn[:, hoff:hoff + T], w_hn, t2[:, T:2 * T],
                                 start=False, stop=True)
                act_zz(znb[:, 2 * i * T:(2 * i + 1) * T], ps_zn[:, 0:T])
                act_n(znb[:, (2 * i + 1) * T:(2 * i + 2) * T],
                      ps_zn[:, hoff:hoff + T])
            # h = cs + zz*(n - cs) over both tiles at [2T] width
            zn4 = znb.rearrange("p (g f) -> p g f", g=2)
            zz_v = zn4[:, :, 0:T]
            n_v = zn4[:, :, T:2 * T]
            cs_v = cs.rearrange("p (g f) -> p g f", g=2)
            u = tpool.tile([HID, 2 * T], BF16, name="u_sb", tag="u")
            u_v = u.rearrange("p (g f) -> p g f", g=2)
            nc.vector.tensor_sub(u_v, n_v, cs_v)
            v = tpool.tile([HID, 2 * T], BF16, name="v_sb", tag="v")
            v_v = v.rearrange("p (g f) -> p g f", g=2)
            nc.vector.tensor_mul(v_v, zz_v, u_v)
            nc.vector.tensor_add(h_lvl[l][:, t0:t0 + 2 * T], v, cs)
            if with_mask:
                mask_mul(h_lvl[l][:, t0:t0 + 2 * T], l, t0, 2 * T)

        def solo_back(st):
            l, t0, T = st["l"], st["t0"], st["T"]
            hoff = max(T, 512)
            t2, cs, xp = st["t2"], st["cs"], st["xp"]
            ps_zn = pspool.tile([HID, 1024], F32, name="ps_zn", tag="psz")
            for s, dst in ((2, ps_zn[:, 0:T]), (3, ps_zn[:, hoff:hoff + T])):
                nc.tensor.matmul(dst, wx[32 * s:32 * (s + 1)],
                                 xp[32 * s:32 * (s + 1)],
                                 start=True, stop=False,
                                 tile_position=(32 * s, 0))
            nc.tensor.matmul(ps_zn[:, 0:T], w_hz, cs, start=False, stop=True)
            nc.tensor.matmul(ps_zn[:, hoff:hoff + T], w_hn, t2[:, 0:T],
                             start=False, stop=False)
            nc.tensor.matmul(ps_zn[:, hoff:hoff + T], w_hn, t2[:, T:2 * T],
                             start=False, stop=True)
            zn = apool.tile([HID, 2 * T], BF16, name="zn_s", tag="act")
            act_zz(zn[:, 0:T], ps_zn[:, 0:T])
            act_n(zn[:, T:2 * T], ps_zn[:, hoff:hoff + T])
            u = tpool.tile([HID, T], BF16, name="u_s", tag="u")
            v = tpool.tile([HID, T], BF16, name="v_s", tag="v")
            # blend in halves: the first half is the next level's planeL, so
            # finishing it early lets the next level's hr matmul start while
            # the second half still computes.
            halves = ((0, T // 2), (T // 2, T)) if T >= 64 else ((0, T),)
            for a, b in halves:
                nc.vector.tensor_sub(u[:, a:b], zn[:, T + a:T + b],
                                     cs[:, a:b])
                nc.vector.tensor_mul(v[:, a:b], zn[:, a:b], u[:, a:b])
                nc.vector.tensor_add(h_lvl[l][:, t0 + a:t0 + b],
                                     v[:, a:b], cs[:, a:b])
            if with_mask:
                mask_mul(h_lvl[l][:, t0:t0 + T], l, t0, T)

        # ---------------- emission ----------------
        # units: leaf pairs interleaved 2:1 with L13 tile-pairs (L13 pair j
        # needs exactly leaf pairs 2j, 2j+1), then levels 12..9.
        units = [("leaf", k) for k in range(n_pairs)]
        for l in range(DEPTH - 2, 8, -1):
            for j in range(_R(l) // T_TILE // 2):
                units.append(("pair", l, j))

        # unskewed per-unit emission: engine queue depth provides overlap;
        # PSUM WAR deps always point at already-emitted instructions.
        for ui, u in enumerate(units):
            if ui == 2:
                nc.sync.dma_start(out=xsmall, in_=xsmall_d.ap())
                nc.sync.dma_start(out=w_out, in_=w_out_d.ap())
            if u[0] == "leaf":
                leaf_back(leaf_front(u[1]))
            else:
                l, j = u[1], u[2]
                stA = int_front(l, 2 * j, T_TILE, cs_width=2 * T_TILE)
                stB = int_front(l, 2 * j + 1, T_TILE)
                pair_back(stA, stB)

        # sequential tail: levels 8..0
        for l in range(8, -1, -1):
            T = min(T_TILE, _R(l))
            for k in range(_R(l) // T):
                solo_back(int_front(l, k, T, cs_width=T, split_r=True))

        # ---------------- output head ----------------
        h0f = tpool.tile([HID, B_LOCAL], F32, name="h0f", tag="h0f")
        nc.vector.tensor_copy(h0f, h_lvl[0])
        ps_out = pspool.tile([HID, B_LOCAL], F32, name="ps_out", tag="psr")
        nc.tensor.matmul(ps_out, w_out, h0f, start=True, stop=True)
        out_sb = opool.tile([HID, B_LOCAL], F32, name="out_sb", tag="out_sb")
        if with_bias:
            nc.scalar.activation(out_sb, ps_out,
                                 mybir.ActivationFunctionType.Identity,
                                 bias=b_out)
        else:
            nc.scalar.copy(out_sb, ps_out)
        nc.sync.dma_start(out=out_d.ap(), in_=out_sb)

    nc.compile()
    return nc


def host_prep(inputs, with_mask=False, with_bias=False):
    t = np.ascontiguousarray(np.asarray(inputs["targets"], np.float32))
    N = t.shape[0]
    assert N == 2**DEPTH - 1 and t.shape[2] == IN_DIM
    leaf = DEPTH - 1

    xt = np.ascontiguousarray(t.transpose(2, 0, 1)).astype(BF16NP)
    revs = {l: _bitrev(l) for l in range(DEPTH)}

    def plain_t(w):
        return np.ascontiguousarray(np.asarray(w, np.float32).T).astype(BF16NP)

    w_ir = plain_t(inputs["W_ir"])
    w_iz = plain_t(inputs["W_iz"])
    w_in = plain_t(inputs["W_in"])
    w_out = np.ascontiguousarray(
        np.concatenate([np.asarray(inputs["W_mu"], np.float32),
                        np.asarray(inputs["W_lv"], np.float32)], axis=0).T)

    wcat = np.zeros((128, 5 * HID), BF16NP)
    wcat[:, 0 * HID:1 * HID] = plain_t(inputs["W_hr"])
    wcat[:, 1 * HID:2 * HID] = plain_t(inputs["W_hz"])
    wcat[:, 2 * HID:3 * HID] = plain_t(inputs["W_hn"])
    for i, wsrc in enumerate((w_ir, w_ir, w_iz, w_in)):
        wcat[32 * i:32 * (i + 1), 3 * HID:4 * HID] = wsrc
    for i, wsrc in enumerate((w_iz, w_in, w_iz, w_in)):
        wcat[32 * i:32 * (i + 1), 4 * HID:5 * HID] = wsrc

    shared = dict(wcat=wcat, w_out=w_out)
    if with_bias:
        b = {k: np.asarray(inputs[k], np.float32) for k in
             ("b_ir", "b_hr", "b_iz", "b_hz", "b_in", "b_hn", "b_mu", "b_lv")}
        bias = np.zeros((HID, 4), np.float32)
        bias[:, 0] = b["b_ir"] + b["b_hr"]
        bias[:, 1] = -(b["b_iz"] + b["b_hz"])
        bias[:, 2] = b["b_in"] + b["b_hn"]
        bias[:128, 3] = np.concatenate([b["b_mu"], b["b_lv"]])
        shared["biases"] = bias

    int_lvls = list(range(DEPTH - 2, SMALL_MAX_LVL, -1))
    n_pairs = _R(leaf) // (2 * T_TILE)

    in_maps = []
    for c in range(N_CORES):
        b0 = c * B_LOCAL
        xc = xt[:, :, b0:b0 + B_LOCAL]
        xl = {}
        for l in range(DEPTH):
            start = 2**l - 1
            blk = xc[:, start + revs[l], :]
            xl[l] = np.ascontiguousarray(blk.reshape(IN_DIM, _R(l)))

        xint = np.concatenate([np.tile(xl[l], (4, 1)) for l in int_lvls],
                              axis=1)
        half = _R(leaf) // 2
        lblocks = []
        for k in range(n_pairs):
            xA = xl[leaf][:, k * T_TILE:(k + 1) * T_TILE]
            xB = xl[leaf][:, half + k * T_TILE:half + (k + 1) * T_TILE]
            lblocks.append(np.concatenate([xA, xA, xB, xB], axis=0))
        xleaf = np.concatenate(lblocks, axis=1)
        xsmall = np.concatenate([np.tile(xl[l], (4, 1))
                                 for l in range(SMALL_MAX_LVL, -1, -1)],
                                axis=1)
        m = dict(shared)
        m["xint"] = np.ascontiguousarray(xint)
        m["xleaf"] = np.ascontiguousarray(xleaf)
        m["xsmall"] = np.ascontiguousarray(xsmall)
        if with_mask:
            mk = np.asarray(inputs["mask"], np.float32)[:, b0:b0 + B_LOCAL]
            mblocks = []
            for l in range(DEPTH):
                start = 2**l - 1
                mblocks.append(mk[start + revs[l], :].reshape(1, _R(l)))
            mcat = np.concatenate(mblocks, axis=1)
            m["mask_bc"] = np.ascontiguousarray(
                np.broadcast_to(mcat, (HID, mcat.shape[1]))).astype(BF16NP)
        in_maps.append(m)
    return in_maps


_PROGRAM_CACHE = {}


def _get_program(with_mask, with_bias):
    key = (with_mask, with_bias)
    if key not in _PROGRAM_CACHE:
        _PROGRAM_CACHE[key] = build_program(with_mask=with_mask,
                                            with_bias=with_bias)
    return _PROGRAM_CACHE[key]


def run_on_device(inputs, trace=False, **trace_kw):
    with_mask = not np.all(np.asarray(inputs["mask"]) == 1.0)
    with_bias = any(
        np.any(np.asarray(inputs[k]) != 0.0)
        for k in ("b_ir", "b_hr", "b_iz", "b_hz", "b_in", "b_hn",
                  "b_mu", "b_lv"))
    nc = _get_program(with_mask, with_bias)
    in_maps = host_prep(inputs, with_mask=with_mask, with_bias=with_bias)
    res = bass_utils.run_bass_kernel_spmd(
        nc, in_maps, core_ids=list(range(N_CORES)), trace=trace, **trace_kw)
    mu = np.zeros((BATCH, OUT_DIM), np.float32)
    lv = np.zeros((BATCH, OUT_DIM), np.float32)
    for c in range(N_CORES):
        o = res.results[c]["out"]
        mu[c * B_LOCAL:(c + 1) * B_LOCAL] = o[:OUT_DIM].T
        lv[c * B_LOCAL:(c + 1) * B_LOCAL] = o[OUT_DIM:].T
    return (mu, lv), res


def kernel(**inputs):
    (mu, lv), _ = run_on_device(inputs)
    return mu, lv


# revision 23
# speedup vs baseline: 1.0015x; 1.0015x over previous
"""Trainium2 Bass kernel for nn_Encoder_55490977464569 (binary-tree GRU encoder).

Strategy (v3)
-------------
Data-parallel over batch: B=16 -> 2 batch columns per NeuronCore, zero
collectives. Each core runs its whole tree (32767 nodes) leaves->root with all
hidden states resident in SBUF (bf16); only `targets` is streamed in.

Layout: feature-major [128 features (partitions), node*batch columns], each
level's nodes stored in BIT-REVERSED in-level order. With bit-reversal at
every level, the children of parent tile [t0, t0+T) are planeL =
child[:, t0:t0+T] and planeR = child[:, R_parent+t0 : +T] (both contiguous),
and the parent's h is written back contiguously. All elementwise ops run on
packed bf16 SBUF operands (DVE 2x mode); STT (1x) and gpsimd (SBUF port
contention) are avoided entirely -- measured on HW: TT=2x, TENSOR_SCALAR=4x,
STT=1x, gpsimd TT inflates concurrent DVE ops 3.6x.

Gates use Sigmoid ACTs (plain weights); 1-z comes free via ACT scale=-1:
zz = sigmoid(-z_pre). Per interior tile pair (2 tiles of T parents):
  - per tile: one [128,T] x DMA (replicated 4x across partition strips);
    4 tile_position-packed K=32 matmuls run concurrently seeding
    ps_r[0:T], ps_r[T:2T], ps_z, ps_n; W_hr @ hl|hr accumulates into ps_r;
    one Sigmoid ACT [2T] -> r; one TT [2T] with a two-region child AP
    computes t2 = r * h_child for both children.
  - per pair: cs = hl+hr as one [2T] TT; W_hz @ cs and W_hn @ t2l + t2r
    accumulate into ps_zn; zz = sigmoid(-ps_z) and n = tanh(ps_n) ACTs;
    blend h = cs + zz*(n - cs) as three [2T]-wide TTs over two-region APs.
Emission is software-pipelined one unit deep so the PE streams continuously.
"""

import sys

if "/opt/trn_rl_repo" not in sys.path:
    sys.path.insert(0, "/opt/trn_rl_repo")
if "/opt/trn_rl_repo/concourse" not in sys.path:
    sys.path.insert(0, "/opt/trn_rl_repo/concourse")

import numpy as np
import ml_dtypes

from concourse import bass, mybir, tile, bacc
from concourse import bass_utils

BF16NP = ml_dtypes.bfloat16
F32 = mybir.dt.float32
BF16 = mybir.dt.bfloat16

N_CORES = 8
DEPTH = 15
HID = 128
IN_DIM = 32
OUT_DIM = 64
BATCH = 16
B_LOCAL = BATCH // N_CORES

T_TILE = 512
SMALL_MAX_LVL = 7

ADD = mybir.AluOpType.add
SUB = mybir.AluOpType.subtract
MULT = mybir.AluOpType.mult
TANH = mybir.ActivationFunctionType.Tanh
SIGM = mybir.ActivationFunctionType.Sigmoid


def _R(l):
    return 2**l * B_LOCAL


def _bitrev(n_bits):
    n = 1 << n_bits
    p = np.zeros(n, dtype=np.int64)
    for i in range(n):
        r = 0
        x = i
        for _ in range(n_bits):
            r = (r << 1) | (x & 1)
            x >>= 1
        p[i] = r
    return p


def build_program(with_mask=False, with_bias=False):
    nc = bacc.Bacc("TRN2", target_bir_lowering=False, debug=False,
                   num_devices=1)
    leaf = DEPTH - 1

    int_lvls = list(range(DEPTH - 2, SMALL_MAX_LVL, -1))
    int_off = {}
    off = 0
    for l in int_lvls:
        int_off[l] = off
        off += _R(l)
    xint_d = nc.dram_tensor("xint", [128, off], BF16, kind="ExternalInput")
    n_pairs = _R(leaf) // (2 * T_TILE)
    xleaf_d = nc.dram_tensor("xleaf", [128, n_pairs * T_TILE], BF16,
                             kind="ExternalInput")
    small_cols = sum(_R(l) for l in range(SMALL_MAX_LVL + 1))
    xsmall_d = nc.dram_tensor("xsmall", [128, small_cols], BF16,
                              kind="ExternalInput")
    wcat_d = nc.dram_tensor("wcat", [128, 5 * HID], BF16, kind="ExternalInput")
    w_out_d = nc.dram_tensor("w_out", [HID, 2 * OUT_DIM], F32,
                             kind="ExternalInput")
    out_d = nc.dram_tensor("out", [HID, B_LOCAL], F32, kind="ExternalOutput")
    if with_bias:
        bias_d = nc.dram_tensor("biases", [HID, 4], F32, kind="ExternalInput")
    if with_mask:
        total_z = sum(_R(l) for l in range(DEPTH))
        mask_d = nc.dram_tensor("mask_bc", [HID, total_z], BF16,
                                kind="ExternalInput")
        mask_off = {}
        moff = 0
        for l in range(DEPTH):
            mask_off[l] = moff
            moff += _R(l)

    from contextlib import ExitStack
    with tile.TileContext(nc) as tc, ExitStack() as stack:
        consts = stack.enter_context(tc.tile_pool(name="consts", bufs=1))
        hpool = stack.enter_context(tc.tile_pool(name="hpool", bufs=1))
        xpool = stack.enter_context(tc.tile_pool(name="xpool", bufs=6))
        apool = stack.enter_context(tc.tile_pool(name="apool", bufs=4))
        tpool = stack.enter_context(tc.tile_pool(name="tpool", bufs=4))
        pspool = stack.enter_context(tc.tile_pool(name="pspool", bufs=2,
                                                  space="PSUM"))
        opool = stack.enter_context(tc.tile_pool(name="opool", bufs=1))

        wcat_sb = consts.tile([128, 5 * HID], BF16, name="wcat_sb",
                              tag="wcat_sb")
        nc.sync.dma_start(out=wcat_sb, in_=wcat_d.ap())
        w_hr = wcat_sb[:, 0 * HID:1 * HID]
        w_hz = wcat_sb[:, 1 * HID:2 * HID]
        w_hn = wcat_sb[:, 2 * HID:3 * HID]
        wx = wcat_sb[:, 3 * HID:4 * HID]   # [w_ir; w_ir; w_iz; w_in]
        wl = wcat_sb[:, 4 * HID:5 * HID]   # [w_iz; w_in; w_iz; w_in]
        w_out = consts.tile([HID, 2 * OUT_DIM], F32, name="w_out_sb",
                            tag="w_out_sb")
        xsmall = consts.tile([128, small_cols], BF16, name="xsmall",
                             tag="xsmall")
        # (their DMAs are emitted after the first leaf units, off the
        # startup critical path)
        small_off = {}
        soff = 0
        for l in range(SMALL_MAX_LVL, -1, -1):
            small_off[l] = soff
            soff += _R(l)
        if with_bias:
            bias_sb = consts.tile([HID, 4], F32, name="bias_sb", tag="bias_sb")
            nc.sync.dma_start(out=bias_sb, in_=bias_d.ap())
            b_r = bias_sb[:, 0:1]
            b_zneg = bias_sb[:, 1:2]   # -(b_iz + b_hz)
            b_n = bias_sb[:, 2:3]
            b_out = bias_sb[:, 3:4]

        h_lvl = [hpool.tile([HID, _R(l)], BF16, name=f"h_{l}", tag=f"h_{l}")
                 for l in range(DEPTH)]

        def mask_mul(view, lvl, col0, width):
            m_sb = tpool.tile([HID, width], BF16, name="m_sb", tag="m_sb")
            nc.sync.dma_start(
                out=m_sb,
                in_=mask_d.ap()[:, mask_off[lvl] + col0:
                                mask_off[lvl] + col0 + width])
            nc.vector.tensor_mul(view, view, m_sb)

        def act_zz(dst, src):
            if with_bias:
                nc.scalar.activation(dst, src, SIGM, bias=b_zneg, scale=-1.0)
            else:
                nc.scalar.activation(dst, src, SIGM, scale=-1.0)

        def act_n(dst, src):
            if with_bias:
                nc.scalar.activation(dst, src, TANH, bias=b_n)
            else:
                nc.scalar.activation(dst, src, TANH)

        # ---------------- leaf pair-tiles ----------------
        def leaf_front(k):
            T = T_TILE
            xp = xpool.tile([128, T], BF16, name="xp_leaf", tag="xp")
            nc.sync.dma_start(out=xp, in_=xleaf_d.ap()[:, k * T:(k + 1) * T])
            psA = pspool.tile([HID, 1024], F32, name="psA", tag="psr")
            psB = pspool.tile([HID, 1024], F32, name="psB", tag="psz")
            for s, dst in enumerate((psA[:, 0:T], psA[:, 512:512 + T],
                                     psB[:, 0:T], psB[:, 512:512 + T])):
                nc.tensor.matmul(dst, wl[32 * s:32 * (s + 1)],
                                 xp[32 * s:32 * (s + 1)],
                                 start=True, stop=True,
                                 tile_position=(32 * s, 0))
            znA = apool.tile([HID, 2 * T], BF16, name="znA", tag="act")
            znB = apool.tile([HID, 2 * T], BF16, name="znB", tag="act")
            for ps, zn in ((psA, znA), (psB, znB)):
                act_zz(zn[:, 0:T], ps[:, 0:T])
                act_n(zn[:, T:2 * T], ps[:, 512:512 + T])
            return {"znA": znA, "znB": znB, "k": k}

        def leaf_back(st):
            T = T_TILE
            k = st["k"]
            znA, znB = st["znA"], st["znB"]
            hv = h_lvl[leaf]
            half = _R(leaf) // 2
            nc.vector.tensor_mul(hv[:, k * T:(k + 1) * T],
                                 znA[:, 0:T], znA[:, T:2 * T])
            nc.vector.tensor_mul(hv[:, half + k * T:half + (k + 1) * T],
                                 znB[:, 0:T], znB[:, T:2 * T])
            if with_mask:
                mask_mul(hv[:, k * T:(k + 1) * T], leaf, k * T, T)
                mask_mul(hv[:, half + k * T:half + (k + 1) * T], leaf,
                         half + k * T, T)

        # ---------------- interior tiles ----------------
        def int_front(l, k, T, cs_width=0, split_r=False):
            """r-phase of one tile. cs_width>0: also emit the cs add
            (hl+hr) covering [t0, t0+cs_width) -- off the critical chain,
            it only needs the child level."""
            t0 = k * T
            ch = h_lvl[l + 1]
            R = _R(l)
            hoff = max(T, 512)
            hl = ch[:, t0:t0 + T]
            hr = ch[:, R + t0:R + t0 + T]
            if l > SMALL_MAX_LVL:
                xp = xpool.tile([128, T], BF16, name="xp_int", tag="xp")
                nc.sync.dma_start(
                    out=xp, in_=xint_d.ap()[:, int_off[l] + t0:
                                            int_off[l] + t0 + T])
            else:
                xp = xsmall[:, small_off[l] + t0:small_off[l] + t0 + T]
            cs = None
            if cs_width:
                cs = tpool.tile([HID, cs_width], BF16, name="cs", tag="cs")
                nc.vector.tensor_add(cs, ch[:, t0:t0 + cs_width],
                                     ch[:, R + t0:R + t0 + cs_width])
            ps_r = pspool.tile([HID, 1024], F32, name="ps_r", tag="psr")
            for s, dst in ((0, ps_r[:, 0:T]), (1, ps_r[:, hoff:hoff + T])):
                nc.tensor.matmul(dst, wx[32 * s:32 * (s + 1)],
                                 xp[32 * s:32 * (s + 1)],
                                 start=True, stop=False,
                                 tile_position=(32 * s, 0))
            nc.tensor.matmul(ps_r[:, 0:T], w_hr, hl, start=False, stop=True)
            nc.tensor.matmul(ps_r[:, hoff:hoff + T], w_hr, hr,
                             start=False, stop=True)
            r_sb = apool.tile([HID, 2 * T], BF16, name="r_sb", tag="act")
            t2 = tpool.tile([HID, 2 * T], BF16, name="t2", tag="t2")
            kw = dict(bias=b_r) if with_bias else {}
            if T == hoff and not split_r:
                nc.scalar.activation(r_sb, ps_r, SIGM, **kw)
                # t2 = r * h_child, both children via one 2-region child AP
                ch2 = ch.rearrange("p (g f) -> p g f", g=2)[:, :, t0:t0 + T]
                nc.vector.tensor_mul(t2.rearrange("p (g f) -> p g f", g=2),
                                     r_sb.rearrange("p (g f) -> p g f", g=2),
                                     ch2)
            else:
                # latency-split: t2l can proceed while rr still activates
                nc.scalar.activation(r_sb[:, 0:T], ps_r[:, 0:T], SIGM, **kw)
                nc.vector.tensor_mul(t2[:, 0:T], r_sb[:, 0:T], hl)
                nc.scalar.activation(r_sb[:, T:2 * T],
                                     ps_r[:, hoff:hoff + T], SIGM, **kw)
                nc.vector.tensor_mul(t2[:, T:2 * T], r_sb[:, T:2 * T], hr)
            return {"l": l, "t0": t0, "T": T, "t2": t2, "xp": xp, "cs": cs}

        def pair_back(stA, stB):
            """zn-phase for two adjacent tiles (t0 of B = t0 of A + T)."""
            l, T = stA["l"], stA["T"]
            t0 = stA["t0"]
            hoff = max(T, 512)
            cs = stA["cs"]
            znb = apool.tile([HID, 4 * T], BF16, name="znb", tag="znb")
            for i, st in enumerate((stA, stB)):
                t2, xp = st["t2"], st["xp"]
                ps_zn = pspool.tile([HID, 1024], F32, name="ps_zn", tag="psz")
                for s, dst in ((2, ps_zn[:, 0:T]),
                               (3, ps_zn[:, hoff:hoff + T])):
                    nc.tensor.matmul(dst, wx[32 * s:32 * (s + 1)],
                                     xp[32 * s:32 * (s + 1)],
                                     start=True, stop=False,
                                     tile_position=(32 * s, 0))
                nc.tensor.matmul(ps_zn[:, 0:T], w_hz, cs[:, i * T:(i + 1) * T],
                                 start=False, stop=True)
                nc.tensor.matmul(ps_zn[:, hoff:hoff + T], w_hn, t2[:, 0:T],
                                 start=False, stop=False)
                nc.tensor.matmul(ps_zn[:, hoff:hoff + T], w_hn, t2[:, T:2 * T],
                                 start=False, stop=True)
                act_zz(znb[:, 2 * i * T:(2 * i + 1) * T], ps_zn[:, 0:T])
                act_n(znb[:, (2 * i + 1) * T:(2 * i + 2) * T],
                      ps_zn[:, hoff:hoff + T])
            # h = cs + zz*(n - cs) over both tiles at [2T] width
            zn4 = znb.rearrange("p (g f) -> p g f", g=2)
            zz_v = zn4[:, :, 0:T]
            n_v = zn4[:, :, T:2 * T]
            cs_v = cs.rearrange("p (g f) -> p g f", g=2)
            u = tpool.tile([HID, 2 * T], BF16, name="u_sb", tag="u")
            u_v = u.rearrange("p (g f) -> p g f", g=2)
            nc.vector.tensor_sub(u_v, n_v, cs_v)
            v = tpool.tile([HID, 2 * T], BF16, name="v_sb", tag="v")
            v_v = v.rearrange("p (g f) -> p g f", g=2)
            nc.vector.tensor_mul(v_v, zz_v, u_v)
            nc.vector.tensor_add(h_lvl[l][:, t0:t0 + 2 * T], v, cs)
            if with_mask:
                mask_mul(h_lvl[l][:, t0:t0 + 2 * T], l, t0, 2 * T)

        def solo_back(st):
            l, t0, T = st["l"], st["t0"], st["T"]
            hoff = max(T, 512)
            t2, cs, xp = st["t2"], st["cs"], st["xp"]
            ps_zn = pspool.tile([HID, 1024], F32, name="ps_zn", tag="psz")
            for s, dst in ((2, ps_zn[:, 0:T]), (3, ps_zn[:, hoff:hoff + T])):
                nc.tensor.matmul(dst, wx[32 * s:32 * (s + 1)],
                                 xp[32 * s:32 * (s + 1)],
                                 start=True, stop=False,
                                 tile_position=(32 * s, 0))
            nc.tensor.matmul(ps_zn[:, 0:T], w_hz, cs, start=False, stop=True)
            nc.tensor.matmul(ps_zn[:, hoff:hoff + T], w_hn, t2[:, 0:T],
                             start=False, stop=False)
            nc.tensor.matmul(ps_zn[:, hoff:hoff + T], w_hn, t2[:, T:2 * T],
                             start=False, stop=True)
            zn = apool.tile([HID, 2 * T], BF16, name="zn_s", tag="act")
            act_zz(zn[:, 0:T], ps_zn[:, 0:T])
            act_n(zn[:, T:2 * T], ps_zn[:, hoff:hoff + T])
            u = tpool.tile([HID, T], BF16, name="u_s", tag="u")
            v = tpool.tile([HID, T], BF16, name="v_s", tag="v")
            nc.vector.tensor_sub(u, zn[:, T:2 * T], cs)
            nc.vector.tensor_mul(v, zn[:, 0:T], u)
            nc.vector.tensor_add(h_lvl[l][:, t0:t0 + T], v, cs)
            if with_mask:
                mask_mul(h_lvl[l][:, t0:t0 + T], l, t0, T)

        # ---------------- emission ----------------
        # units: leaf pairs interleaved 2:1 with L13 tile-pairs (L13 pair j
        # needs exactly leaf pairs 2j, 2j+1), then levels 12..9.
        units = [("leaf", k) for k in range(n_pairs)]
        for l in range(DEPTH - 2, 8, -1):
            for j in range(_R(l) // T_TILE // 2):
                units.append(("pair", l, j))

        # unskewed per-unit emission: engine queue depth provides overlap;
        # PSUM WAR deps always point at already-emitted instructions.
        for ui, u in enumerate(units):
            if ui == 2:
                nc.sync.dma_start(out=xsmall, in_=xsmall_d.ap())
                nc.sync.dma_start(out=w_out, in_=w_out_d.ap())
            if u[0] == "leaf":
                leaf_back(leaf_front(u[1]))
            else:
                l, j = u[1], u[2]
                stA = int_front(l, 2 * j, T_TILE, cs_width=2 * T_TILE)
                stB = int_front(l, 2 * j + 1, T_TILE)
                pair_back(stA, stB)

        # sequential tail: levels 8..0
        for l in range(8, -1, -1):
            T = min(T_TILE, _R(l))
            for k in range(_R(l) // T):
                solo_back(int_front(l, k, T, cs_width=T, split_r=True))

        # ---------------- output head ----------------
        h0f = tpool.tile([HID, B_LOCAL], F32, name="h0f", tag="h0f")
        nc.vector.tensor_copy(h0f, h_lvl[0])
        ps_out = pspool.tile([HID, B_LOCAL], F32, name="ps_out", tag="psr")
        nc.tensor.matmul(ps_out, w_out, h0f, start=True, stop=True)
        out_sb = opool.tile([HID, B_LOCAL], F32, name="out_sb", tag="out_sb")
        if with_bias:
            nc.scalar.activation(out_sb, ps_out,
                                 mybir.ActivationFunctionType.Identity,
                                 bias=b_out)
        else:
            nc.scalar.copy(out_sb, ps_out)
        nc.sync.dma_start(out=out_d.ap(), in_=out_sb)

    nc.compile()
    return nc


def host_prep(inputs, with_mask=False, with_bias=False):
    t = np.ascontiguousarray(np.asarray(inputs["targets"], np.float32))
    N = t.shape[0]
    assert N == 2**DEPTH - 1 and t.shape[2] == IN_DIM
    leaf = DEPTH - 1

    xt = np.ascontiguousarray(t.transpose(2, 0, 1)).astype(BF16NP)
    revs = {l: _bitrev(l) for l in range(DEPTH)}

    def plain_t(w):
        return np.ascontiguousarray(np.asarray(w, np.float32).T).astype(BF16NP)

    w_ir = plain_t(inputs["W_ir"])
    w_iz = plain_t(inputs["W_iz"])
    w_in = plain_t(inputs["W_in"])
    w_out = np.ascontiguousarray(
        np.concatenate([np.asarray(inputs["W_mu"], np.float32),
                        np.asarray(inputs["W_lv"], np.float32)], axis=0).T)

    wcat = np.zeros((128, 5 * HID), BF16NP)
    wcat[:, 0 * HID:1 * HID] = plain_t(inputs["W_hr"])
    wcat[:, 1 * HID:2 * HID] = plain_t(inputs["W_hz"])
    wcat[:, 2 * HID:3 * HID] = plain_t(inputs["W_hn"])
    for i, wsrc in enumerate((w_ir, w_ir, w_iz, w_in)):
        wcat[32 * i:32 * (i + 1), 3 * HID:4 * HID] = wsrc
    for i, wsrc in enumerate((w_iz, w_in, w_iz, w_in)):
        wcat[32 * i:32 * (i + 1), 4 * HID:5 * HID] = wsrc

    shared = dict(wcat=wcat, w_out=w_out)
    if with_bias:
        b = {k: np.asarray(inputs[k], np.float32) for k in
             ("b_ir", "b_hr", "b_iz", "b_hz", "b_in", "b_hn", "b_mu", "b_lv")}
        bias = np.zeros((HID, 4), np.float32)
        bias[:, 0] = b["b_ir"] + b["b_hr"]
        bias[:, 1] = -(b["b_iz"] + b["b_hz"])
        bias[:, 2] = b["b_in"] + b["b_hn"]
        bias[:128, 3] = np.concatenate([b["b_mu"], b["b_lv"]])
        shared["biases"] = bias

    int_lvls = list(range(DEPTH - 2, SMALL_MAX_LVL, -1))
    n_pairs = _R(leaf) // (2 * T_TILE)

    in_maps = []
    for c in range(N_CORES):
        b0 = c * B_LOCAL
        xc = xt[:, :, b0:b0 + B_LOCAL]
        xl = {}
        for l in range(DEPTH):
            start = 2**l - 1
            blk = xc[:, start + revs[l], :]
            xl[l] = np.ascontiguousarray(blk.reshape(IN_DIM, _R(l)))

        xint = np.concatenate([np.tile(xl[l], (4, 1)) for l in int_lvls],
                              axis=1)
        half = _R(leaf) // 2
        lblocks = []
        for k in range(n_pairs):
            xA = xl[leaf][:, k * T_TILE:(k + 1) * T_TILE]
            xB = xl[leaf][:, half + k * T_TILE:half + (k + 1) * T_TILE]
            lblocks.append(np.concatenate([xA, xA, xB, xB], axis=0))
        xleaf = np.concatenate(lblocks, axis=1)
        xsmall = np.concatenate([np.tile(xl[l], (4, 1))
                                 for l in range(SMALL_MAX_LVL, -1, -1)],
                                axis=1)
        m = dict(shared)
        m["xint"] = np.ascontiguousarray(xint)
        m["xleaf"] = np.ascontiguousarray(xleaf)
        m["xsmall"] = np.ascontiguousarray(xsmall)
        if with_mask:
            mk = np.asarray(inputs["mask"], np.float32)[:, b0:b0 + B_LOCAL]
            mblocks = []
            for l in range(DEPTH):
                start = 2**l - 1
                mblocks.append(mk[start + revs[l], :].reshape(1, _R(l)))
            mcat = np.concatenate(mblocks, axis=1)
            m["mask_bc"] = np.ascontiguousarray(
                np.broadcast_to(mcat, (HID, mcat.shape[1]))).astype(BF16NP)
        in_maps.append(m)
    return in_maps


_PROGRAM_CACHE = {}


def _get_program(with_mask, with_bias):
    key = (with_mask, with_bias)
    if key not in _PROGRAM_CACHE:
        _PROGRAM_CACHE[key] = build_program(with_mask=with_mask,
                                            with_bias=with_bias)
    return _PROGRAM_CACHE[key]


def run_on_device(inputs, trace=False, **trace_kw):
    with_mask = not np.all(np.asarray(inputs["mask"]) == 1.0)
    with_bias = any(
        np.any(np.asarray(inputs[k]) != 0.0)
        for k in ("b_ir", "b_hr", "b_iz", "b_hz", "b_in", "b_hn",
                  "b_mu", "b_lv"))
    nc = _get_program(with_mask, with_bias)
    in_maps = host_prep(inputs, with_mask=with_mask, with_bias=with_bias)
    res = bass_utils.run_bass_kernel_spmd(
        nc, in_maps, core_ids=list(range(N_CORES)), trace=trace, **trace_kw)
    mu = np.zeros((BATCH, OUT_DIM), np.float32)
    lv = np.zeros((BATCH, OUT_DIM), np.float32)
    for c in range(N_CORES):
        o = res.results[c]["out"]
        mu[c * B_LOCAL:(c + 1) * B_LOCAL] = o[:OUT_DIM].T
        lv[c * B_LOCAL:(c + 1) * B_LOCAL] = o[OUT_DIM:].T
    return (mu, lv), res


def kernel(**inputs):
    (mu, lv), _ = run_on_device(inputs)
    return mu, lv
